# revision 24
# baseline (speedup 1.0000x reference)
"""Trainium2 Bass kernel for a transformer decoder block.

Shapes (hardcoded): B=4, S=1024, D=1024, H=16 heads, DH=64, FFN F=4096.

Sharding: 8 cores = 4 batches x 2 sequence-halves.  Core (b, h) handles
query rows {64*(2t+h)+r : t in 0..7, r in 0..63} of batch b (interleaved
64-row blocks so the causal-attention work per core is identical -> one
uniform SPMD program).  Each core recomputes the (small) K/V projections
it needs, so no collectives are required.

On-chip layout is feature-major ("transposed"): activations live as
[feature, token] so every matmul contraction sits on the partition axis.
The host pre-transposes inputs/weights and re-transposes the output.

Softmax denominators ride along in the AV matmuls: the staged V tiles
carry an extra ones column per head-half, so each AV accumulation also
produces the row-sum of the probability tile in a spare PSUM row (no
separate ones-vector matmuls).

LN1/LN2 scale+shift are folded host-side into the consumer weights
(ca_wq / ff_w1), so the bf16 activations the next stage needs come
straight out of the (x-mean)*rstd multiply; the f32 residual copies are
produced by deferred ScalarE affines off the critical path.

Scheduling: engines execute their instruction streams in order, so each
attention head-pair's softmax (ScalarE-bound) is emitted with "filler"
projection matmul groups for the next head pair woven between its
k-chunks, keeping the PE busy while exps drain.
"""

import sys

if "/opt/trn_rl_repo" not in sys.path:
    sys.path.insert(0, "/opt/trn_rl_repo")

import numpy as np
import ml_dtypes

B, S, D, H, F, DH = 4, 1024, 1024, 16, 4096, 64
NCORES = 8
SQ = 512            # query rows per core
NDT = D // 128      # 8 d-tiles
NFT = F // 128      # 32 f-tiles
NHP = H // 2        # 8 head pairs
NKC = S // 128      # 8 k chunks
BF16 = ml_dtypes.bfloat16

_PROG = None


def _build_program():
    import concourse.mybir as mybir
    from concourse import bacc
    from concourse.tile import TileContext

    f32 = mybir.dt.float32
    bf16 = mybir.dt.bfloat16
    AF = mybir.ActivationFunctionType
    OP = mybir.AluOpType

    nc = bacc.Bacc("TRN2", target_bir_lowering=False, debug=False,
                   num_devices=NCORES)

    def din(name, shape, dt=bf16):
        return nc.dram_tensor(name, shape, dt, kind="ExternalInput")

    # activations, partition-major so each loads with ONE contiguous DMA
    xt_full = din("xt_full", [128, NDT, S])          # X^T (K/V source)
    xq = din("xq", [128, NDT, SQ])                   # X^T own q rows
    xr = din("xr", [128, NDT, SQ], f32)              # residual (f32)
    enc_t = din("enc_t", [128, NDT, S])              # encoder^T
    sa_mask = din("sa_mask", [128, NKC, 64])         # causal boundary slabs

    # weights staged host-side in exactly the sbuf tile layout
    w_sa_q = din("w_sa_q", [NHP, 128, NDT, 128])
    w_sa_k = din("w_sa_k", [NHP, 128, NDT, 128])
    w_sa_v = din("w_sa_v", [2, 128, NDT, 512])
    w_sa_o = din("w_sa_o", [NDT, 128, NDT, 128])
    w_ca_q = din("w_ca_q", [NHP, 128, NDT, 128])
    w_ca_k = din("w_ca_k", [NHP, 128, NDT, 128])
    w_ca_v = din("w_ca_v", [2, 128, NDT, 512])
    w_ca_o = din("w_ca_o", [NDT, 128, NDT, 128])
    w_ff1 = din("w_ff1", [NFT, 128, NDT, 128])
    w_ff2 = din("w_ff2", [NDT, 128, NFT, 128])

    # all small per-feature vectors concatenated: one DMA
    # cols: bq1 0:8 | bq2 8:16 | bo1 16:24 | bo2 24:32 | b2 32:40 |
    #       ln1g 40:48 | ln1b 48:56 | ln2g .. | ln3b 72:88 | b1 88:120
    NV = 120
    v_all = din("v_all", [128, NV], f32)

    out_t = nc.dram_tensor("out_t", [NDT, 128, SQ], f32, kind="ExternalOutput")

    with TileContext(nc) as tc:
        with tc.tile_pool(name="p_acc", bufs=2, space="PSUM") as p_acc, \
             tc.tile_pool(name="p_s", bufs=2, space="PSUM") as p_s, \
             tc.tile_pool(name="p_pav", bufs=2, space="PSUM") as p_pav, \
             tc.tile_pool(name="const", bufs=1) as cpool, \
             tc.tile_pool(name="big", bufs=1) as big, \
             tc.tile_pool(name="wcol", bufs=6) as wcol, \
             tc.tile_pool(name="wbig", bufs=2) as wbig, \
             tc.tile_pool(name="pt", bufs=6) as ptp, \
             tc.tile_pool(name="bc", bufs=3) as bcp, \
             tc.tile_pool(name="sm", bufs=1) as smp, \
             tc.tile_pool(name="tmp", bufs=3) as tmpp, \
             tc.tile_pool(name="outp", bufs=2) as outp:

            # ------------- startup: activation DMAs first -------------
            XQ = big.tile([128, NDT, SQ], bf16, tag="outb")
            nc.sync.dma_start(out=XQ[:, 0:4, :], in_=xq[:, 0:4, :])
            wqt0 = wcol.tile([128, NDT, 128], bf16, tag="wcol")
            nc.sync.dma_start(out=wqt0[:], in_=w_sa_q[0])
            nc.sync.dma_start(out=XQ[:, 4:8, :], in_=xq[:, 4:8, :])

            # ---------------- constants / small vectors ----------------
            # LN stat matmuls use 1/D so psum rows are mean / E[x^2]
            oneD16 = cpool.tile([128, 1], bf16)
            nc.vector.memset(oneD16[:], 1.0 / D)
            eps_t = cpool.tile([1, 1], f32)
            nc.vector.memset(eps_t[:], 1e-12)

            VA = cpool.tile([128, NV], f32)
            nc.sync.dma_start(out=VA[:], in_=v_all[:])
            bq1_sb, bq2_sb = VA[:, 0:8], VA[:, 8:16]
            bo1_sb, bo2_sb = VA[:, 16:24], VA[:, 24:32]
            b2_sb = VA[:, 32:40]
            ln_sb = {j: (VA[:, 40 + 16 * (j - 1):48 + 16 * (j - 1)],
                         VA[:, 48 + 16 * (j - 1):56 + 16 * (j - 1)])
                     for j in (1, 2, 3)}
            b1_sb = VA[:, 88:120]

            MS = cpool.tile([128, NKC, 64], bf16)
            nc.sync.dma_start(out=MS[:], in_=sa_mask[:])

            XT = big.tile([128, NDT, S], bf16, tag="xt")

            # ---------------- filler-step builders ----------------
            # Each returned closure emits one psum matmul group; they are
            # woven between attention k-chunks (and LN-tail iterations) to
            # keep the PE fed while ScalarE/DVE work drains.
            def q_steps(hp, wq_d, src_q, bq_sb, QT, wpre=None):
                def run():
                    if wpre is None:
                        wqt = wcol.tile([128, NDT, 128], bf16, tag="wcol")
                        nc.sync.dma_start(out=wqt[:], in_=wq_d[hp])
                    else:
                        wqt = wpre
                    pq = p_acc.tile([128, SQ], f32, tag="acc")
                    for dt in range(NDT):
                        nc.tensor.matmul(pq[:], wqt[:, dt, :], src_q[:, dt, :],
                                         start=(dt == 0), stop=(dt == NDT - 1))
                    nc.vector.tensor_scalar_add(QT[:, hp, :], pq[:],
                                                bq_sb[:, hp:hp + 1])
                return [run]

            def k_steps(hp, wk_d, src_kv, KT, wpre=None):
                cell = {"w": wpre}

                def run_kh(kh):
                    def run():
                        if kh == 0 and cell["w"] is None:
                            cell["w"] = wcol.tile([128, NDT, 128], bf16,
                                                  tag="wcol", name="wkt")
                            nc.sync.dma_start(out=cell["w"][:], in_=wk_d[hp])
                        wkt = cell["w"]
                        pk = p_acc.tile([128, 512], f32, tag="acc")
                        for dt in range(NDT):
                            nc.tensor.matmul(
                                pk[:], wkt[:, dt, :],
                                src_kv[:, dt, 512 * kh:512 * (kh + 1)],
                                start=(dt == 0), stop=(dt == NDT - 1))
                        nc.vector.tensor_copy(
                            KT[:, hp, 512 * kh:512 * (kh + 1)], pk[:])
                    return run
                return [run_kh(0), run_kh(1)]

            def v_steps(g, wv_d, src_kv, V2):
                cell = {}

                def run_kc(kc):
                    def run():
                        if kc == 0:
                            cell["w"] = wbig.tile([128, NDT, 512], bf16,
                                                  tag="wbig", name="wvt")
                            nc.sync.dma_start(out=cell["w"][:], in_=wv_d[g])
                        wvt = cell["w"]
                        pv = p_acc.tile([128, 4, 2, 64], f32, tag="acc")
                        for dt in range(NDT):
                            nc.tensor.matmul(
                                pv[:, :, :, :],
                                src_kv[:, dt, 128 * kc:128 * (kc + 1)],
                                wvt[:, dt, :],
                                start=(dt == 0), stop=(dt == NDT - 1))
                        nc.vector.tensor_copy(
                            V2[:, kc, 4 * g:4 * g + 4, 0:64], pv[:, :, 0, :])
                        nc.vector.tensor_copy(
                            V2[:, kc, 4 * g:4 * g + 4, 65:129], pv[:, :, 1, :])
                    return run
                return [run_kc(kc) for kc in range(NKC)]

            def attention(hp, QT, KT, V2, ATTN, causal, fillers=()):
                # both halves: AV in psum rows 0:64, softmax denom in row
                # 64 (the staged V tiles carry a trailing ones column).
                # The denom recips are lifted to partition 0 by a gpsimd
                # DMA (broadcast only reads partition 0 on HW); the final
                # normalize muls are returned as a closure and emitted
                # inside the NEXT attention so the DVE stream never waits
                # on the gpsimd/DMA chain.
                pav_a = p_pav.tile([128, SQ], f32, tag="pav_a")
                pav_b = p_pav.tile([128, SQ], f32, tag="pav_b")
                fillers = list(fillers)
                fi = 0
                for j in range(NKC):
                    n0 = 64 * j if causal else 0
                    sa_ = p_s.tile([128, SQ], f32, tag="s")
                    sb_ = p_s.tile([128, SQ], f32, tag="s")
                    ks = slice(128 * j, 128 * (j + 1))
                    nc.tensor.matmul(sa_[:, n0:SQ], KT[0:64, hp, ks],
                                     QT[0:64, hp, n0:SQ], start=True, stop=True)
                    nc.tensor.matmul(sb_[:, n0:SQ], KT[64:128, hp, ks],
                                     QT[64:128, hp, n0:SQ], start=True,
                                     stop=True)
                    pta = ptp.tile([128, SQ], bf16, tag="pt")
                    ptb = ptp.tile([128, SQ], bf16, tag="pt")
                    nc.scalar.activation(out=pta[:, n0:SQ], in_=sa_[:, n0:SQ],
                                         func=AF.Exp, scale=0.125)
                    nc.scalar.activation(out=ptb[:, n0:SQ], in_=sb_[:, n0:SQ],
                                         func=AF.Exp, scale=0.125)
                    if causal:
                        nc.vector.tensor_mul(pta[:, n0:n0 + 64],
                                             pta[:, n0:n0 + 64], MS[:, j, :])
                        nc.vector.tensor_mul(ptb[:, n0:n0 + 64],
                                             ptb[:, n0:n0 + 64], MS[:, j, :])
                    # fillers go HERE (between scores and AV) so the PE chews
                    # on them while ScalarE exps this chunk
                    while fi < len(fillers) and fi * NKC < (j + 1) * len(fillers):
                        fillers[fi]()
                        fi += 1
                    st, sp = (j == 0), (j == NKC - 1)
                    nc.tensor.matmul(pav_a[0:65, n0:SQ],
                                     V2[:, j, hp, 0:65],
                                     pta[:, n0:SQ], start=st, stop=sp)
                    nc.tensor.matmul(pav_b[0:65, n0:SQ],
                                     V2[:, j, hp, 65:130],
                                     ptb[:, n0:SQ], start=st, stop=sp)
                while fi < len(fillers):
                    fillers[fi]()
                    fi += 1
                cz = smp.tile([65, 2, SQ], f32, tag="cz")
                nc.vector.tensor_copy(cz[64:65, 0, :], pav_a[64:65, :])
                nc.vector.tensor_copy(cz[64:65, 1, :], pav_b[64:65, :])
                # unnormalized AV to SBUF: frees the psum banks immediately
                # so the next attention's AV never waits on this tail
                av_a = tmpp.tile([64, SQ], bf16, tag="av", bufs=2)
                nc.vector.tensor_copy(av_a[0:64, :], pav_a[0:64, :])
                av_b = tmpp.tile([64, SQ], bf16, tag="av", bufs=2)
                nc.vector.tensor_copy(av_b[0:64, :], pav_b[0:64, :])
                # gpsimd-issued partition-0 lift (own DMA queue)
                nc.gpsimd.dma_start(out=cz[0:1, :, :], in_=cz[64:65, :, :])
                RA = bcp.tile([128, SQ], f32, tag="bc")
                RB = bcp.tile([128, SQ], f32, tag="bc")

                def fin1():
                    # emitted inside the NEXT attention: the lift DMA is
                    # long done, so these never stall the DVE stream
                    rda = smp.tile([1, SQ], f32, tag="m1", name="rda")
                    rdb = smp.tile([1, SQ], f32, tag="sq1", name="rdb")
                    nc.vector.reciprocal_approx_fast(out=rda[:],
                                                     in_=cz[0:1, 0, :])
                    nc.vector.reciprocal_approx_fast(out=rdb[:],
                                                     in_=cz[0:1, 1, :])
                    nc.gpsimd.partition_broadcast(RA[:], rda[:])
                    nc.gpsimd.partition_broadcast(RB[:], rdb[:])

                def fin2():
                    nc.vector.tensor_mul(ATTN[0:64, hp, :], av_a[0:64, :],
                                         RA[0:64, :])
                    tb = ptp.tile([64, SQ], bf16, tag="pt", name="tb")
                    nc.vector.tensor_mul(tb[0:64, :], av_b[0:64, :],
                                         RB[0:64, :])
                    nc.gpsimd.dma_start(out=ATTN[64:128, hp, :],
                                        in_=tb[0:64, :])
                return fin1, fin2

            def ln_tail(pst1, pst2, y, ln_g, ln_b, out_bf, out_f32, dma_out,
                        fillers=(), per_dt_hook=None):
                fillers = list(fillers)
                m1 = smp.tile([1, SQ], f32, tag="m1")
                nc.vector.tensor_copy(m1[:], pst1[0:1, :])  # mean (ones=1/D)
                MB = bcp.tile([128, SQ], f32, tag="bc")
                nc.gpsimd.partition_broadcast(MB[:], m1[:])
                sq1 = smp.tile([1, SQ], f32, tag="sq1")
                nc.vector.tensor_mul(sq1[:], m1[:], m1[:])
                varp = smp.tile([1, SQ], f32, tag="varp")
                nc.vector.tensor_sub(varp[:], pst2[0:1, :], sq1[:])
                sv = smp.tile([1, SQ], f32, tag="sv")
                nc.scalar.activation(out=sv[:], in_=varp[:], func=AF.Sqrt,
                                     bias=eps_t[:], scale=float(D) / (D - 1))
                rstd = smp.tile([1, SQ], f32, tag="rstd")
                nc.vector.reciprocal_approx_fast(out=rstd[:], in_=sv[:])
                RS = bcp.tile([128, SQ], f32, tag="bc")
                nc.gpsimd.partition_broadcast(RS[:], rstd[:])
                fi = 0
                for dt in range(NDT):
                    # in-place: y is dead after the tail (stats already
                    # accumulated from the bf16 copies)
                    nc.vector.tensor_sub(y[:, dt, :], y[:, dt, :], MB[:])
                    if dma_out is not None:
                        # LN3: full affine, alternating ACT/DVE so neither
                        # engine backlogs the final DMAs
                        nc.vector.tensor_mul(y[:, dt, :], y[:, dt, :], RS[:])
                        od = outp.tile([128, SQ], f32, tag="od")
                        if dt % 2 == 0:
                            nc.scalar.activation(out=od[:], in_=y[:, dt, :],
                                                 func=AF.Identity,
                                                 bias=ln_b[:, dt:dt + 1],
                                                 scale=ln_g[:, dt:dt + 1])
                        else:
                            nc.vector.tensor_scalar(od[:], y[:, dt, :],
                                                    ln_g[:, dt:dt + 1],
                                                    ln_b[:, dt:dt + 1],
                                                    OP.mult, OP.add)
                        nc.sync.dma_start(out=dma_out[dt], in_=od[:])
                    else:
                        # affine folded into consumer weights: bf16 "hat"
                        # output comes straight from the rstd multiply
                        nc.vector.tensor_mul(out_bf[:, dt, :], y[:, dt, :],
                                             RS[:])
                        if per_dt_hook is not None:
                            per_dt_hook(dt)
                    if fi < len(fillers):
                        fillers[fi]()
                        fi += 1
                while fi < len(fillers):
                    fillers[fi]()
                    fi += 1
                if dma_out is None:
                    # deferred residual: f32 affine from the bf16 hat values
                    # (ScalarE, off the critical path)
                    for dt in range(NDT):
                        nc.scalar.activation(out=out_f32[:, dt, :],
                                             in_=out_bf[:, dt, :],
                                             func=AF.Identity,
                                             bias=ln_b[:, dt:dt + 1],
                                             scale=ln_g[:, dt:dt + 1])

            def proj_ln(wo_d, ATTN, bo_sb, resid, ln_g, ln_b, y_tag,
                        out_bf=None, out_f32=None, dma_out=None,
                        proj_fillers=(), tail_fillers=(), per_dt_hook=None,
                        pre=()):
                """wo projection + residual + layernorm (feature-major)."""
                for p in pre:
                    p()
                proj_fillers = list(proj_fillers)
                y = big.tile([128, NDT, SQ], f32, tag=y_tag)
                pst1 = p_pav.tile([128, SQ], f32, tag="pav_a")
                pst2 = p_pav.tile([128, SQ], f32, tag="pav_b")
                fi = 0
                for dt in range(NDT):
                    wot = wcol.tile([128, NDT, 128], bf16, tag="wcol")
                    nc.sync.dma_start(out=wot[:], in_=wo_d[dt])
                    po = p_acc.tile([128, SQ], f32, tag="acc")
                    for ht in range(NDT):
                        nc.tensor.matmul(po[:], wot[:, ht, :], ATTN[:, ht, :],
                                         start=(ht == 0), stop=(ht == NDT - 1))
                    nc.vector.scalar_tensor_tensor(
                        out=y[:, dt, :], in0=po[:], scalar=bo_sb[:, dt:dt + 1],
                        in1=resid[:, dt, :], op0=OP.add, op1=OP.add)
                    yb = tmpp.tile([128, SQ], bf16, tag="yb", bufs=2)
                    nc.vector.tensor_copy(yb[:], y[:, dt, :])
                    sq = tmpp.tile([128, SQ], bf16, tag="sq", bufs=2)
                    nc.vector.tensor_mul(sq[:], yb[:], yb[:])
                    nc.tensor.matmul(pst1[0:1, :], oneD16[:, 0:1], yb[:],
                                     start=(dt == 0), stop=(dt == NDT - 1))
                    nc.tensor.matmul(pst2[0:1, :], oneD16[:, 0:1], sq[:],
                                     start=(dt == 0), stop=(dt == NDT - 1))
                    if fi < len(proj_fillers):
                        proj_fillers[fi]()
                        fi += 1
                while fi < len(proj_fillers):
                    proj_fillers[fi]()
                    fi += 1
                ln_tail(pst1, pst2, y, ln_g, ln_b, out_bf, out_f32, dma_out,
                        fillers=tail_fillers, per_dt_hook=per_dt_hook)

            # ================= self-attention =================
            QT = big.tile([128, NHP, SQ], bf16, tag="qt")
            KT = big.tile([128, NHP, S], bf16, tag="kt")
            V2 = big.tile([128, NKC, NHP, 130], bf16, tag="v2")
            for kc in range(NKC):
                nc.vector.memset(V2[:, kc, :, 64:65], 1.0)
                nc.vector.memset(V2[:, kc, :, 129:130], 1.0)
            ATTN = big.tile([128, NDT, SQ], bf16, tag="attn")

            def sa_steps(hp):
                st = q_steps(hp, w_sa_q, XQ, bq1_sb, QT,
                             wpre=(wqt0 if hp == 0 else None)) \
                    + k_steps(hp, w_sa_k, XT, KT)
                if hp % 4 == 0:
                    st += v_steps(hp // 4, w_sa_v, XT, V2)
                return st

            steps0 = sa_steps(0)
            steps0[0]()                        # Q(0) needs only XQ + wq
            nc.sync.dma_start(out=XT[:, 0:4, :], in_=xt_full[:, 0:4, :])
            nc.sync.dma_start(out=XT[:, 4:8, :], in_=xt_full[:, 4:8, :])
            for step in steps0[1:]:
                step()
            # encoder activations: no deps, load during SA attention
            ENC = big.tile([128, NDT, S], bf16, tag="enc")
            nc.sync.dma_start(out=ENC[:], in_=enc_t[:])
            fin = None
            for hp in range(1, NHP):
                st = sa_steps(hp)
                fl = ([fin[0]] + st[:2] + [fin[1]] + st[2:]) if fin else st
                fin = attention(hp - 1, QT, KT, V2, ATTN, True, fl)
            fin = attention(NHP - 1, QT, KT, V2, ATTN, True,
                            list(fin) if fin else [])

            XR = big.tile([128, NDT, SQ], f32, tag="resid")
            nc.sync.dma_start(out=XR[:], in_=xr[:])
            OUT1B = big.tile([128, NDT, SQ], bf16, tag="outb")
            OUT1F = big.tile([128, NDT, SQ], f32, tag="resid")

            # CA K/V for head-pair 0 weave into the LN1 projection+tail
            # (they only need ENC); Q(0) needs OUT1B so it runs last.
            KT2 = big.tile([128, NHP, S], bf16, tag="kt")
            V2c = big.tile([128, NKC, NHP, 130], bf16, tag="v2")
            for kc in range(NKC):
                nc.vector.memset(V2c[:, kc, :, 64:65], 1.0)
                nc.vector.memset(V2c[:, kc, :, 129:130], 1.0)
            QT2 = big.tile([128, NHP, SQ], bf16, tag="qt")
            ATTN2 = big.tile([128, NDT, SQ], bf16, tag="attn")

            ca0_kv = k_steps(0, w_ca_k, ENC, KT2) \
                + v_steps(0, w_ca_v, ENC, V2c)
            ca0_q = q_steps(0, w_ca_q, OUT1B, bq2_sb, QT2)
            # prefetch CA hp=1 weights ahead of the O-proj DMA burst
            wkt1 = wcol.tile([128, NDT, 128], bf16, tag="wcol")
            nc.sync.dma_start(out=wkt1[:], in_=w_ca_k[1])
            wqt1 = wcol.tile([128, NDT, 128], bf16, tag="wcol")
            nc.sync.dma_start(out=wqt1[:], in_=w_ca_q[1])
            ca1_pre = {1: (wkt1, wqt1)}

            proj_ln(w_sa_o, ATTN, bo1_sb, XR, ln_sb[1][0], ln_sb[1][1],
                    y_tag="y", out_bf=OUT1B, out_f32=OUT1F,
                    proj_fillers=ca0_kv[:8], tail_fillers=ca0_kv[8:],
                    pre=list(fin))
            ca0_q[0]()   # CA Q(0): needs all of OUT1B, so after the tail

            # ================= cross-attention =================
            def ca_steps(hp):
                st = k_steps(hp, w_ca_k, ENC, KT2,
                             wpre=ca1_pre.get(hp, (None,))[0])
                if hp % 4 == 0:
                    st += v_steps(hp // 4, w_ca_v, ENC, V2c)
                st += q_steps(hp, w_ca_q, OUT1B, bq2_sb, QT2,
                              wpre=ca1_pre.get(hp, (None, None))[1])
                return st

            fin = None
            for hp in range(1, NHP):
                st = ca_steps(hp)
                fl = ([fin[0]] + st[:2] + [fin[1]] + st[2:]) if fin else st
                fin = attention(hp - 1, QT2, KT2, V2c, ATTN2, False, fl)
            fin = attention(NHP - 1, QT2, KT2, V2c, ATTN2, False,
                            list(fin) if fin else [])

            OUT2B = big.tile([128, NDT, SQ], bf16, tag="outb")
            OUT2F = big.tile([128, NDT, SQ], f32, tag="resid")

            # first two FFN-w1 chains accumulate per-dt during the LN2 tail
            H1 = big.tile([128, NFT, SQ], bf16, tag="xt")  # reuse XT slot
            w1t_pre = []
            ph_pre = []

            def prep_w1_chains():
                for ft in range(2):
                    w1t = wcol.tile([128, NDT, 128], bf16, tag="w1pre",
                                    bufs=2)
                    nc.sync.dma_start(out=w1t[:], in_=w_ff1[ft])
                    w1t_pre.append(w1t)
                    # score banks are free once attention is done
                    ph_pre.append(p_s.tile([128, SQ], f32, tag="s",
                                           name="ph_pre"))

            def w1_hook(dt):
                for ft in range(2):
                    nc.tensor.matmul(ph_pre[ft][:], w1t_pre[ft][:, dt, :],
                                     OUT2B[:, dt, :],
                                     start=(dt == 0), stop=(dt == NDT - 1))

            prep_w1_chains()
            proj_ln(w_ca_o, ATTN2, bo2_sb, OUT1F, ln_sb[2][0], ln_sb[2][1],
                    y_tag="y", out_bf=OUT2B, out_f32=OUT2F,
                    per_dt_hook=w1_hook, pre=list(fin))

            # ================= feed-forward =================
            for ft in range(NFT):
                if ft < 2:
                    ph = ph_pre[ft]
                else:
                    w1t = wcol.tile([128, NDT, 128], bf16, tag="w1pre",
                                    bufs=2)
                    nc.sync.dma_start(out=w1t[:], in_=w_ff1[ft])
                    ph = p_acc.tile([128, SQ], f32, tag="acc")
                    for dt in range(NDT):
                        nc.tensor.matmul(ph[:], w1t[:, dt, :], OUT2B[:, dt, :],
                                         start=(dt == 0), stop=(dt == NDT - 1))
                nc.scalar.activation(out=H1[:, ft, :], in_=ph[:], func=AF.Relu,
                                     bias=b1_sb[:, ft:ft + 1], scale=1.0)

            y3 = big.tile([128, NDT, SQ], f32, tag="y")
            pst1 = p_pav.tile([128, SQ], f32, tag="pav_a")
            pst2 = p_pav.tile([128, SQ], f32, tag="pav_b")
            for dt in range(NDT):
                w2t = wbig.tile([128, NFT, 128], bf16, tag="wbig")
                nc.sync.dma_start(out=w2t[:], in_=w_ff2[dt])
                pf = p_acc.tile([128, SQ], f32, tag="acc")
                for ft in range(NFT):
                    nc.tensor.matmul(pf[:], w2t[:, ft, :], H1[:, ft, :],
                                     start=(ft == 0), stop=(ft == NFT - 1))
                nc.vector.scalar_tensor_tensor(
                    out=y3[:, dt, :], in0=pf[:], scalar=b2_sb[:, dt:dt + 1],
                    in1=OUT2F[:, dt, :], op0=OP.add, op1=OP.add)
                yb = tmpp.tile([128, SQ], bf16, tag="yb", bufs=2)
                nc.vector.tensor_copy(yb[:], y3[:, dt, :])
                sq = tmpp.tile([128, SQ], bf16, tag="sq", bufs=2)
                nc.vector.tensor_mul(sq[:], yb[:], yb[:])
                nc.tensor.matmul(pst1[0:1, :], oneD16[:, 0:1], yb[:],
                                 start=(dt == 0), stop=(dt == NDT - 1))
                nc.tensor.matmul(pst2[0:1, :], oneD16[:, 0:1], sq[:],
                                 start=(dt == 0), stop=(dt == NDT - 1))
            ln_tail(pst1, pst2, y3, ln_sb[3][0], ln_sb[3][1], None, None,
                    out_t)

    nc.compile()
    return nc


def _qrows(h):
    return np.concatenate(
        [np.arange(64 * (2 * t + h), 64 * (2 * t + h) + 64) for t in range(8)])


def _prepare_in_maps(inputs):
    f = np.float32
    di = np.asarray(inputs["decoder_input"], f)
    eo = np.asarray(inputs["encoder_output"], f)
    mask = np.asarray(inputs["mask"])

    def b16(a):
        return np.ascontiguousarray(a).astype(BF16)

    def wmat(w):  # (H, D, DH) -> (D, H*DH)
        return np.transpose(np.asarray(w, f), (1, 0, 2)).reshape(D, H * DH)

    def colmajor(w, no, co):  # [D_in, N_out] -> [no, 128, D_in/128, co]
        return w.reshape(w.shape[0] // 128, 128, no, co).transpose(2, 1, 0, 3)

    def pmajor(xt, n):  # [D, n] (feature-major) -> [128, NDT, n]
        return np.ascontiguousarray(
            xt.reshape(NDT, 128, n).transpose(1, 0, 2))

    ln1_g = np.asarray(inputs["ln1_g"], f)
    ln1_b = np.asarray(inputs["ln1_b"], f)
    ln2_g = np.asarray(inputs["ln2_g"], f)
    ln2_b = np.asarray(inputs["ln2_b"], f)

    shared = {}
    vecs = {}
    for p in ("sa", "ca"):
        wq = wmat(inputs[f"{p}_wq"])
        bq = np.asarray(inputs[f"{p}_bq"], f).reshape(H * DH)
        if p == "ca":
            # LN1 affine folded into the cross-attention Q projection
            bq = bq + ln1_b @ wq
            wq = ln1_g[:, None] * wq
        shared[f"w_{p}_q"] = b16(colmajor(wq, NHP, 128))
        shared[f"w_{p}_k"] = b16(colmajor(wmat(inputs[f"{p}_wk"]), NHP, 128))
        shared[f"w_{p}_v"] = b16(colmajor(wmat(inputs[f"{p}_wv"]), 2, 512))
        wo = np.asarray(inputs[f"{p}_wo"], f)
        shared[f"w_{p}_o"] = b16(colmajor(wo, NDT, 128))
        vecs[f"bq_{p}"] = bq
        bv = np.asarray(inputs[f"{p}_bv"], f).reshape(H * DH)
        vecs[f"bo_{p}"] = np.asarray(inputs[f"{p}_bo"], f) + bv @ wo
    # LN2 affine folded into the FFN first layer
    w1 = np.asarray(inputs["ff_w1"], f)
    b1 = np.asarray(inputs["ff_b1"], f) + ln2_b @ w1
    w1 = ln2_g[:, None] * w1
    shared["w_ff1"] = b16(colmajor(w1, NFT, 128))
    shared["w_ff2"] = b16(colmajor(np.asarray(inputs["ff_w2"], f), NDT, 128))

    def cols(v, n):  # [n*128] -> [128, n]
        return np.asarray(v, f).reshape(n, 128).T

    va = np.concatenate([
        cols(vecs["bq_sa"], NHP), cols(vecs["bq_ca"], NHP),
        cols(vecs["bo_sa"], NDT), cols(vecs["bo_ca"], NDT),
        cols(inputs["ff_b2"], NDT),
        cols(ln1_g, NDT), cols(ln1_b, NDT),
        cols(ln2_g, NDT), cols(ln2_b, NDT),
        cols(inputs["ln3_g"], NDT), cols(inputs["ln3_b"], NDT),
        cols(b1, NFT),
    ], axis=1)
    shared["v_all"] = np.ascontiguousarray(va, dtype=f)

    qr = {h: _qrows(h) for h in (0, 1)}
    in_maps = []
    for c in range(NCORES):
        b, h = divmod(c, 2)
        X = di[b]
        m = dict(shared)
        m["xt_full"] = b16(pmajor(X.T, S))
        Xq = X[qr[h]]
        m["xq"] = b16(pmajor(Xq.T, SQ))
        m["xr"] = np.ascontiguousarray(pmajor(Xq.T, SQ), dtype=f)
        m["enc_t"] = b16(pmajor(eo[b].T, S))
        mb = mask[b][qr[h]].astype(f)          # [SQ q, S k]
        slabs = np.zeros((NKC, 128, 64), f)
        for j in range(NKC):
            slabs[j] = mb[64 * j:64 * j + 64, 128 * j:128 * (j + 1)].T
        m["sa_mask"] = np.ascontiguousarray(
            slabs.transpose(1, 0, 2)).astype(BF16)
        in_maps.append(m)
    return in_maps


def _collect_output(results):
    qr = {h: _qrows(h) for h in (0, 1)}
    out = np.zeros((B, S, D), np.float32)
    for c in range(NCORES):
        b, h = divmod(c, 2)
        ot = np.asarray(results[c]["out_t"], np.float32).reshape(D, SQ)
        out[b, qr[h]] = ot.T
    return out


def kernel(**inputs):
    global _PROG
    if _PROG is None:
        _PROG = _build_program()
    from concourse.bass_utils import run_bass_kernel_spmd

    in_maps = _prepare_in_maps(inputs)
    res = run_bass_kernel_spmd(_PROG, in_maps, list(range(NCORES)))
    if res.exec_time_ns is not None:
        print(f"HW exec time: {res.exec_time_ns} ns")
    return _collect_output(res.results)
